# revision 9
# baseline (speedup 1.0000x reference)
"""AQT int8 symmetric-quantized dot_general (bmk,kn->bmn) on 8 TRN2 NeuronCores.

Problem: lhs [2, 4096, 4096] f32, rhs [4096, 4096] f32.
  q_l, s_l = absmax-int8-quantize(lhs, axis=K)   (per-row scales)
  q_r, s_r = absmax-int8-quantize(rhs, axis=K)   (per-col scales)
  out = (q_l @ q_r) * s_l * s_r                  [2, 4096, 4096] f32

Sharding: 2 (batch) x 4 (N columns) grid over 8 cores; K replicated.
Each core computes an independent [4096, 1024] output block - no collectives.

Per-core kernel (Tile framework):
  - rhs pass 1: stream k-tiles, accumulate per-partition |max|, then
    gpsimd partition_all_reduce -> per-column amax (exact f32).
  - rhs pass 2: re-stream k-tiles, q_r = round(rhs * (127/amax)) via the
    fp32 magic-number trick (+1.5*2^23, -1.5*2^23 => round-half-even,
    exactly matching jnp.round), stored as bf16 (int8 values are exact).
  - lhs per m-tile of 128 rows: free-axis amax reduce, quantize+round the
    same way, PE-transpose 128x128 blocks to get K on partitions, then
    32 accumulating matmuls per 512-wide output panel (bf16 x bf16 -> f32).
  - epilogue: out = (psum * s_l) * s_r fused in one DVE op, DMA out.
"""

import os

import numpy as np

import concourse.bass as bass
import concourse.mybir as mybir
import concourse.tile as tile
from concourse import bacc, bass_isa
from concourse.bass import ts
from concourse.bass_utils import run_bass_kernel_spmd
from concourse.masks import make_identity

MAGIC = 12582912.0  # 1.5 * 2**23: fp32 add => round-half-even to integer

_LDW_OPT = os.environ.get("KLIB_LDW_OPT", "0") == "1"
_KK_OUTER = os.environ.get("KLIB_KK_OUTER", "0") == "1"

if _LDW_OPT:
    from concourse import bass_utils as _bu

    if not getattr(_bu, "_ldw_patched", False):
        _orig_run_command = _bu.run_command

        def _patched_run_command(argv, **kw):
            argv = [
                "--enable-ldw-opt=true" if a == "--enable-ldw-opt=false" else a
                for a in argv
            ]
            return _orig_run_command(argv, **kw)

        _bu.run_command = _patched_run_command
        _bu._ldw_patched = True

B, M, K, N = 2, 4096, 4096, 4096
GRID_B, GRID_N = 2, 4  # 8 cores
M_LOC, N_LOC = M, N // GRID_N


def build_nc(m_loc=M_LOC, k=K, n_loc=N_LOC, panel=512):
    f32, bf16 = mybir.dt.float32, mybir.dt.bfloat16
    mult, add = mybir.AluOpType.mult, mybir.AluOpType.add
    nk, nm, npan = k // 128, m_loc // 128, n_loc // panel
    nc = bacc.Bacc("TRN2", target_bir_lowering=False, debug=False)
    lhs_d = nc.dram_tensor("lhs", [m_loc, k], f32, kind="ExternalInput")
    rhs_d = nc.dram_tensor("rhs", [k, n_loc], f32, kind="ExternalInput")
    out_d = nc.dram_tensor("out", [m_loc, n_loc], f32, kind="ExternalOutput")

    with tile.TileContext(nc) as tc:
        with (
            tc.tile_pool(name="const", bufs=1) as constp,
            tc.tile_pool(name="qr", bufs=1) as qrp,
            tc.tile_pool(name="rstat", bufs=1) as rstatp,
            tc.tile_pool(name="rio", bufs=4) as riop,
            tc.tile_pool(name="rtmp", bufs=2) as rtmpp,
            tc.tile_pool(name="lio", bufs=2) as liop,
            tc.tile_pool(name="lq", bufs=2) as lqp,
            tc.tile_pool(name="lstat", bufs=8) as lstatp,
            tc.tile_pool(name="eo", bufs=4) as eop,
            tc.tile_pool(name="pout", bufs=3, space="PSUM") as poutp,
        ):
            # ---- rhs pass 1: per-column amax (exact f32) ----
            acc = rstatp.tile([128, n_loc], f32, tag="acc")
            nc.vector.memset(acc[:], 0.0)
            for kk in range(nk):
                rt = riop.tile([128, n_loc], f32, tag="rt")
                nc.sync.dma_start(rt[:], rhs_d[ts(kk, 128), :])
                ra = rtmpp.tile([128, n_loc], f32, tag="ra")
                nc.scalar.activation(ra[:], rt[:], mybir.ActivationFunctionType.Abs)
                nc.vector.tensor_tensor(
                    acc[:], acc[:], ra[:], op=mybir.AluOpType.max
                )
            amax_r = rstatp.tile([128, n_loc], f32, tag="amax_r")
            nc.gpsimd.partition_all_reduce(
                amax_r[:], acc[:], channels=128, reduce_op=bass_isa.ReduceOp.absmax
            )
            inv_r = rstatp.tile([128, n_loc], f32, tag="inv_r")
            nc.vector.reciprocal(inv_r[:], amax_r[:])
            nc.vector.tensor_scalar_mul(inv_r[:], inv_r[:], 127.0)
            s_r = rstatp.tile([128, n_loc], f32, tag="s_r")
            nc.vector.tensor_scalar_mul(s_r[:], amax_r[:], 1.0 / 127.0)

            # lhs m-tile prep: quantize + xbar-transpose -> (qT, s_l)
            def prep_mtile(mi):
                lt = liop.tile([128, k], f32, tag="lt")
                nc.sync.dma_start(lt[:], lhs_d[ts(mi, 128), :])
                am = lstatp.tile([128, 1], f32, tag="am")
                nc.vector.tensor_reduce(
                    am[:],
                    lt[:],
                    axis=mybir.AxisListType.X,
                    op=mybir.AluOpType.max,
                    apply_absolute_value=True,
                )
                inv_l = lstatp.tile([128, 1], f32, tag="invl")
                nc.vector.reciprocal(inv_l[:], am[:])
                nc.vector.tensor_scalar_mul(inv_l[:], inv_l[:], 127.0)
                s_l = lstatp.tile([128, 1], f32, tag="sl")
                nc.vector.tensor_scalar_mul(s_l[:], am[:], 1.0 / 127.0)
                # in-place: lt = lt * inv_l + MAGIC  (rounds to int at the add)
                nc.vector.tensor_scalar(
                    lt[:], lt[:], inv_l[:], MAGIC, op0=mult, op1=add
                )
                qb = lqp.tile([128, k], bf16, tag="qb")
                nc.scalar.activation(
                    qb[:], lt[:], mybir.ActivationFunctionType.Copy, bias=-MAGIC
                )
                qT = lqp.tile([128, k], bf16, tag="qT")
                # one xbar-transpose DMA does all nk 128x128 block transposes:
                # out[p, b, f] = qb[f, b*128 + p]
                nc.scalar.dma_start_transpose(
                    qT[:].rearrange("p (b f) -> p b f", f=128), qb[:]
                )
                return qT, s_l

            def mm_mtile(mi, qT, s_l):
                if _KK_OUTER:
                    pos = [
                        poutp.tile([128, panel], f32, tag=f"po{p}")
                        for p in range(npan)
                    ]
                    for kk in range(nk):
                        for p in range(npan):
                            nc.tensor.matmul(
                                pos[p][:],
                                qT[:, ts(kk, 128)],
                                qr_tiles[kk][:, ts(p, panel)],
                                start=(kk == 0),
                                stop=(kk == nk - 1),
                            )
                    for p in range(npan):
                        eo = eop.tile([128, panel], f32, tag="eo")
                        nc.vector.scalar_tensor_tensor(
                            eo[:], pos[p][:], s_l[:], s_r[:, ts(p, panel)],
                            op0=mult, op1=mult,
                        )
                        nc.sync.dma_start(out_d[ts(mi, 128), ts(p, panel)], eo[:])
                    return
                for p in range(npan):
                    po = poutp.tile([128, panel], f32, tag="po")
                    for kk in range(nk):
                        nc.tensor.matmul(
                            po[:],
                            qT[:, ts(kk, 128)],
                            qr_tiles[kk][:, ts(p, panel)],
                            start=(kk == 0),
                            stop=(kk == nk - 1),
                        )
                    eo = eop.tile([128, panel], f32, tag="eo")
                    nc.vector.scalar_tensor_tensor(
                        eo[:], po[:], s_l[:], s_r[:, ts(p, panel)], op0=mult, op1=mult
                    )
                    nc.sync.dma_start(out_d[ts(mi, 128), ts(p, panel)], eo[:])

            # prep the first lhs tiles BEFORE rhs pass 2 so their DVE/ACT work
            # (and the first matmuls) isn't queued behind all of pass 2
            prepped = {}
            n_pre = min(2, nm)
            for mi in range(n_pre):
                prepped[mi] = prep_mtile(mi)

            # ---- rhs pass 2: quantize via direct f32->int32 (round-half-even)
            qr_tiles = []
            for kk in range(nk):
                rt = riop.tile([128, n_loc], f32, tag="rt")
                nc.sync.dma_start(rt[:], rhs_d[ts(kk, 128), :])
                ru = rtmpp.tile([128, n_loc], mybir.dt.int32, tag="ru")
                nc.vector.tensor_tensor(ru[:], rt[:], inv_r[:], op=mult)
                qr = qrp.tile([128, n_loc], bf16, tag=f"qr{kk}")
                nc.scalar.copy(qr[:], ru[:])
                qr_tiles.append(qr)

            # ---- m-tile loop: matmuls + epilogue, prepping ahead ----
            for mi in range(nm):
                if mi not in prepped:
                    prepped[mi] = prep_mtile(mi)
                qT, s_l = prepped.pop(mi)
                nxt = mi + n_pre
                if nxt < nm and nxt not in prepped:
                    prepped[nxt] = prep_mtile(nxt)
                mm_mtile(mi, qT, s_l)

    nc.compile()
    return nc


def run_shards(nc, lhs_shards, rhs_shards, trace=False, **kw):
    in_maps = [
        {"lhs": np.ascontiguousarray(l), "rhs": np.ascontiguousarray(r)}
        for l, r in zip(lhs_shards, rhs_shards)
    ]
    return run_bass_kernel_spmd(
        nc, in_maps, core_ids=list(range(len(in_maps))), trace=trace, **kw
    )


_NC_CACHE = {}


def get_full_nc():
    if "nc" not in _NC_CACHE:
        _NC_CACHE["nc"] = build_nc()
    return _NC_CACHE["nc"]


def kernel(lhs, rhs):
    lhs = np.ascontiguousarray(np.asarray(lhs, dtype=np.float32))
    rhs = np.ascontiguousarray(np.asarray(rhs, dtype=np.float32))
    assert lhs.shape == (B, M, K) and rhs.shape == (K, N)
    nc = get_full_nc()
    lhs_shards, rhs_shards = [], []
    for c in range(8):
        pi, qi = c // GRID_N, c % GRID_N
        lhs_shards.append(lhs[pi])
        rhs_shards.append(rhs[:, qi * N_LOC : (qi + 1) * N_LOC])
    res = run_shards(nc, lhs_shards, rhs_shards)
    out = np.empty((B, M, N), np.float32)
    for c in range(8):
        pi, qi = c // GRID_N, c % GRID_N
        out[pi, :, qi * N_LOC : (qi + 1) * N_LOC] = res.results[c]["out"]
    return out


if __name__ == "__main__":
    rng = np.random.default_rng(0)
    lhs = rng.standard_normal((B, M, K), dtype=np.float32)
    rhs = rng.standard_normal((K, N), dtype=np.float32)
    out = kernel(lhs=lhs, rhs=rhs)
    print("kernel output:", out.shape, out.dtype)


# revision 10
# speedup vs baseline: 1.0259x; 1.0259x over previous
"""AQT int8 symmetric-quantized dot_general (bmk,kn->bmn) on 8 TRN2 NeuronCores.

Problem: lhs [2, 4096, 4096] f32, rhs [4096, 4096] f32.
  q_l, s_l = absmax-int8-quantize(lhs, axis=K)   (per-row scales)
  q_r, s_r = absmax-int8-quantize(rhs, axis=K)   (per-col scales)
  out = (q_l @ q_r) * s_l * s_r                  [2, 4096, 4096] f32

Sharding: 2 (batch) x 4 (N columns) grid over 8 cores; K replicated.
Each core computes an independent [4096, 1024] output block - no collectives.

Per-core kernel (Tile framework):
  - rhs pass 1: stream k-tiles, accumulate per-partition |max|, then
    gpsimd partition_all_reduce -> per-column amax (exact f32).
  - rhs pass 2: re-stream k-tiles, q_r = round(rhs * (127/amax)) via the
    fp32 magic-number trick (+1.5*2^23, -1.5*2^23 => round-half-even,
    exactly matching jnp.round), stored as bf16 (int8 values are exact).
  - lhs per m-tile of 128 rows: free-axis amax reduce, quantize+round the
    same way, PE-transpose 128x128 blocks to get K on partitions, then
    32 accumulating matmuls per 512-wide output panel (bf16 x bf16 -> f32).
  - epilogue: out = (psum * s_l) * s_r fused in one DVE op, DMA out.
"""

import os

import numpy as np

import concourse.bass as bass
import concourse.mybir as mybir
import concourse.tile as tile
from concourse import bacc, bass_isa
from concourse.bass import ts
from concourse.bass_utils import run_bass_kernel_spmd
from concourse.masks import make_identity

MAGIC = 12582912.0  # 1.5 * 2**23: fp32 add => round-half-even to integer

_LDW_OPT = os.environ.get("KLIB_LDW_OPT", "0") == "1"
_KK_OUTER = os.environ.get("KLIB_KK_OUTER", "0") == "1"

if _LDW_OPT:
    from concourse import bass_utils as _bu

    if not getattr(_bu, "_ldw_patched", False):
        _orig_run_command = _bu.run_command

        def _patched_run_command(argv, **kw):
            argv = [
                "--enable-ldw-opt=true" if a == "--enable-ldw-opt=false" else a
                for a in argv
            ]
            return _orig_run_command(argv, **kw)

        _bu.run_command = _patched_run_command
        _bu._ldw_patched = True

B, M, K, N = 2, 4096, 4096, 4096
GRID_B, GRID_N = 2, 4  # 8 cores
M_LOC, N_LOC = M, N // GRID_N


def build_nc(m_loc=M_LOC, k=K, n_loc=N_LOC, panel=512):
    f32, bf16 = mybir.dt.float32, mybir.dt.bfloat16
    mult, add = mybir.AluOpType.mult, mybir.AluOpType.add
    nk, nm, npan = k // 128, m_loc // 128, n_loc // panel
    nc = bacc.Bacc("TRN2", target_bir_lowering=False, debug=False)
    lhs_d = nc.dram_tensor("lhs", [m_loc, k], f32, kind="ExternalInput")
    rhs_d = nc.dram_tensor("rhs", [k, n_loc], f32, kind="ExternalInput")
    out_d = nc.dram_tensor("out", [m_loc, n_loc], f32, kind="ExternalOutput")

    with tile.TileContext(nc) as tc:
        with (
            tc.tile_pool(name="const", bufs=1) as constp,
            tc.tile_pool(name="qr", bufs=1) as qrp,
            tc.tile_pool(name="rstat", bufs=1) as rstatp,
            tc.tile_pool(name="rio", bufs=4) as riop,
            tc.tile_pool(name="rtmp", bufs=2) as rtmpp,
            tc.tile_pool(name="lio", bufs=2) as liop,
            tc.tile_pool(name="lq", bufs=2) as lqp,
            tc.tile_pool(name="lstat", bufs=8) as lstatp,
            tc.tile_pool(name="eo", bufs=6) as eop,
            tc.tile_pool(name="pout", bufs=4, space="PSUM") as poutp,
        ):
            # ---- rhs pass 1: per-column amax (exact f32) ----
            acc = rstatp.tile([128, n_loc], f32, tag="acc")
            nc.vector.memset(acc[:], 0.0)
            for kk in range(nk):
                rt = riop.tile([128, n_loc], f32, tag="rt")
                nc.sync.dma_start(rt[:], rhs_d[ts(kk, 128), :])
                ra = rtmpp.tile([128, n_loc], f32, tag="ra")
                nc.scalar.activation(ra[:], rt[:], mybir.ActivationFunctionType.Abs)
                nc.vector.tensor_tensor(
                    acc[:], acc[:], ra[:], op=mybir.AluOpType.max
                )
            amax_r = rstatp.tile([128, n_loc], f32, tag="amax_r")
            nc.gpsimd.partition_all_reduce(
                amax_r[:], acc[:], channels=128, reduce_op=bass_isa.ReduceOp.absmax
            )
            inv_r = rstatp.tile([128, n_loc], f32, tag="inv_r")
            nc.vector.reciprocal(inv_r[:], amax_r[:])
            nc.vector.tensor_scalar_mul(inv_r[:], inv_r[:], 127.0)
            s_r = rstatp.tile([128, n_loc], f32, tag="s_r")
            nc.vector.tensor_scalar_mul(s_r[:], amax_r[:], 1.0 / 127.0)

            # lhs m-tile prep: quantize + xbar-transpose -> (qT, s_l)
            def prep_mtile(mi):
                lt = liop.tile([128, k], f32, tag="lt")
                nc.sync.dma_start(lt[:], lhs_d[ts(mi, 128), :])
                am = lstatp.tile([128, 1], f32, tag="am")
                nc.vector.tensor_reduce(
                    am[:],
                    lt[:],
                    axis=mybir.AxisListType.X,
                    op=mybir.AluOpType.max,
                    apply_absolute_value=True,
                )
                inv_l = lstatp.tile([128, 1], f32, tag="invl")
                nc.vector.reciprocal(inv_l[:], am[:])
                nc.vector.tensor_scalar_mul(inv_l[:], inv_l[:], 127.0)
                s_l = lstatp.tile([128, 1], f32, tag="sl")
                nc.vector.tensor_scalar_mul(s_l[:], am[:], 1.0 / 127.0)
                # in-place: lt = lt * inv_l + MAGIC  (rounds to int at the add)
                nc.vector.tensor_scalar(
                    lt[:], lt[:], inv_l[:], MAGIC, op0=mult, op1=add
                )
                qb = lqp.tile([128, k], bf16, tag="qb")
                nc.scalar.activation(
                    qb[:], lt[:], mybir.ActivationFunctionType.Copy, bias=-MAGIC
                )
                qT = lqp.tile([128, k], bf16, tag="qT")
                # one xbar-transpose DMA does all nk 128x128 block transposes:
                # out[p, b, f] = qb[f, b*128 + p]
                nc.sync.dma_start_transpose(
                    qT[:].rearrange("p (b f) -> p b f", f=128), qb[:]
                )
                return qT, s_l

            def mm_mtile(mi, qT, s_l):
                if _KK_OUTER:
                    pos = [
                        poutp.tile([128, panel], f32, tag=f"po{p}")
                        for p in range(npan)
                    ]
                    for kk in range(nk):
                        for p in range(npan):
                            nc.tensor.matmul(
                                pos[p][:],
                                qT[:, ts(kk, 128)],
                                qr_tiles[kk][:, ts(p, panel)],
                                start=(kk == 0),
                                stop=(kk == nk - 1),
                            )
                    for p in range(npan):
                        eo = eop.tile([128, panel], f32, tag="eo")
                        nc.vector.scalar_tensor_tensor(
                            eo[:], pos[p][:], s_l[:], s_r[:, ts(p, panel)],
                            op0=mult, op1=mult,
                        )
                        nc.scalar.dma_start(out_d[ts(mi, 128), ts(p, panel)], eo[:])
                    return
                for p in range(npan):
                    po = poutp.tile([128, panel], f32, tag="po")
                    for kk in range(nk):
                        nc.tensor.matmul(
                            po[:],
                            qT[:, ts(kk, 128)],
                            qr_tiles[kk][:, ts(p, panel)],
                            start=(kk == 0),
                            stop=(kk == nk - 1),
                        )
                    eo = eop.tile([128, panel], f32, tag="eo")
                    nc.vector.scalar_tensor_tensor(
                        eo[:], po[:], s_l[:], s_r[:, ts(p, panel)], op0=mult, op1=mult
                    )
                    nc.scalar.dma_start(out_d[ts(mi, 128), ts(p, panel)], eo[:])

            # prep the first lhs tiles BEFORE rhs pass 2 so their DVE/ACT work
            # (and the first matmuls) isn't queued behind all of pass 2
            prepped = {}
            n_pre = min(2, nm)
            for mi in range(n_pre):
                prepped[mi] = prep_mtile(mi)

            # ---- rhs pass 2: quantize via direct f32->int32 (round-half-even)
            qr_tiles = []
            for kk in range(nk):
                rt = riop.tile([128, n_loc], f32, tag="rt")
                nc.sync.dma_start(rt[:], rhs_d[ts(kk, 128), :])
                ru = rtmpp.tile([128, n_loc], mybir.dt.int32, tag="ru")
                nc.vector.tensor_tensor(ru[:], rt[:], inv_r[:], op=mult)
                qr = qrp.tile([128, n_loc], bf16, tag=f"qr{kk}")
                nc.scalar.copy(qr[:], ru[:])
                qr_tiles.append(qr)

            # ---- m-tile loop: matmuls + epilogue, prepping ahead ----
            for mi in range(nm):
                if mi not in prepped:
                    prepped[mi] = prep_mtile(mi)
                qT, s_l = prepped.pop(mi)
                mm_mtile(mi, qT, s_l)
                nxt = mi + n_pre
                if nxt < nm and nxt not in prepped:
                    prepped[nxt] = prep_mtile(nxt)

    nc.compile()
    return nc


def run_shards(nc, lhs_shards, rhs_shards, trace=False, **kw):
    in_maps = [
        {"lhs": np.ascontiguousarray(l), "rhs": np.ascontiguousarray(r)}
        for l, r in zip(lhs_shards, rhs_shards)
    ]
    return run_bass_kernel_spmd(
        nc, in_maps, core_ids=list(range(len(in_maps))), trace=trace, **kw
    )


_NC_CACHE = {}


def get_full_nc():
    if "nc" not in _NC_CACHE:
        _NC_CACHE["nc"] = build_nc()
    return _NC_CACHE["nc"]


def kernel(lhs, rhs):
    lhs = np.ascontiguousarray(np.asarray(lhs, dtype=np.float32))
    rhs = np.ascontiguousarray(np.asarray(rhs, dtype=np.float32))
    assert lhs.shape == (B, M, K) and rhs.shape == (K, N)
    nc = get_full_nc()
    lhs_shards, rhs_shards = [], []
    for c in range(8):
        pi, qi = c // GRID_N, c % GRID_N
        lhs_shards.append(lhs[pi])
        rhs_shards.append(rhs[:, qi * N_LOC : (qi + 1) * N_LOC])
    res = run_shards(nc, lhs_shards, rhs_shards)
    out = np.empty((B, M, N), np.float32)
    for c in range(8):
        pi, qi = c // GRID_N, c % GRID_N
        out[pi, :, qi * N_LOC : (qi + 1) * N_LOC] = res.results[c]["out"]
    return out


if __name__ == "__main__":
    rng = np.random.default_rng(0)
    lhs = rng.standard_normal((B, M, K), dtype=np.float32)
    rhs = rng.standard_normal((K, N), dtype=np.float32)
    out = kernel(lhs=lhs, rhs=rhs)
    print("kernel output:", out.shape, out.dtype)
